# Initial kernel scaffold
#
"""Trainium2 Bass kernel for nn_DentalAnatomyLoss.

Computes, for segmentation [B=2, C=32, D=64, H=128, W=128] fp32:
  - crown/root ratio loss (per (b,c) sums over d<32 / d>=32)
  - 3D total-variation loss (mean |diff| along w, h, d)
  - returns stack([crown_root, smoothness, total_anatomy]) fp32 [3]

Strategy: pure data-parallel over the 64 (b,c) slices, 8 per NeuronCore.
Each core reduces its 32 MiB shard to a [128, 160] fp32 partial tensor;
the host combines partials into the 3 scalars.

Per-core engine split (memory regime, ~94 us HBM roofline/core):
  - ScalarE: fp32->bf16 cast with fused accum_out (crown/root sums), and
    Abs+accum_out consuming the h-diff matmul output from PSUM.
  - VectorE: the w-diff as one fused scalar_tensor_tensor (out=max(a,b),
    accum_out=sum) reading fp32 directly (the shift-by-one AP is 1x in
    any dtype); the d-diff as an aligned 2x subtract + 4x fused relu-sum.
    The host recovers sum|a-b| = 2*sum(max(a,b)) - sum(a) - sum(b) and
    sum|d| = 2*sum(max(d,0)) - sum(d), with the signed sums telescoping
    to boundary-column sums.
  - TensorE: bidiagonal matmul computes h-diffs (partition axis) in PSUM.
  - DMA: HBM loads only (the SP ring), ~94 us/core at ~360 GB/s.

Pipelining: xb-dependent work (d-diff, h-diff matmul) is emitted one
chunk late so VectorE never waits on the cast; PSUM is two half-chunk
tiles (4 banks each) so TensorE fills one while ScalarE drains the
other; each PSUM drain is deferred past the next fill.
"""

import os

import numpy as np

B, C, D, H, W = 2, 32, 64, 128, 128
NCORES = 8
JPC = (B * C) // NCORES  # (b,c) slices per core
CROWN_ROOT_W = 2.0
SMOOTH_W = 1.5
EXPECTED_RATIO = 1.2

# accumulator column layout in the [128, ACC_COLS] partial tensor
# (one column per chunk = (slice j, half); 16 chunks per core)
ACC_COLS = 160
COL_X = 0  # 16: sum(x) per chunk
COL_DXP = 16  # 16: sum(max(x[...,w], x[...,w+1])) over w-pairs
COL_TXF = 32  # 16: sum over planes of column w=0
COL_TXL = 48  # 16: sum over planes of column w=W-1
COL_DZP = 64  # 16: sum(max(dz,0)), dz = plane[k+1]-plane[k] (in-chunk)
COL_TZF = 80  # 16: sum of first plane of chunk
COL_TZL = 96  # 16: sum of last plane of chunk
COL_DY = 112  # 32: sum|dy| per (chunk, psum-half)
COL_BNDP = 144  # 8: sum(max(a,b)) for the half0/half1 boundary plane pair
# 152:160 unused (zeroed)

_PROG_CACHE: dict = {}
last_exec_time_ns = None  # set by kernel() when tracing is enabled


def _build_program(jpc=JPC, d=D, h=H, w=W, repeat=1, skip=()):
    """Build the (single) SPMD Bass program run identically on all cores.

    repeat>1 wraps the whole compute in a hardware For_i loop (identical
    result, used only for wall-clock timing of the kernel body).
    """
    from contextlib import ExitStack

    import concourse.tile as tile
    from concourse import bacc, mybir

    f32 = mybir.dt.float32
    bf16 = mybir.dt.bfloat16
    AO = mybir.AluOpType
    AF = mybir.ActivationFunctionType

    ndh = d // 2  # planes per chunk; chunks never straddle the crown/root split
    fsz = ndh * w  # free size of one chunk

    nc = bacc.Bacc(
        "TRN2",
        target_bir_lowering=False,
        debug=False,
        enable_asserts=False,
        num_devices=NCORES,
    )
    seg = nc.dram_tensor("seg", [jpc, d, h, w], f32, kind="ExternalInput").ap()
    bd = nc.dram_tensor("bidiag", [h, h], bf16, kind="ExternalInput").ap()
    out = nc.dram_tensor("partials", [h, ACC_COLS], f32, kind="ExternalOutput").ap()

    with tile.TileContext(nc) as tc, ExitStack() as ctx:
        singles = ctx.enter_context(tc.tile_pool(name="singles", bufs=1))
        x32p = ctx.enter_context(tc.tile_pool(name="x32", bufs=3))
        xbp = ctx.enter_context(tc.tile_pool(name="xb", bufs=4))
        dxp = ctx.enter_context(tc.tile_pool(name="dx", bufs=2))
        dzp = ctx.enter_context(tc.tile_pool(name="dz", bufs=2))
        tinyp = ctx.enter_context(tc.tile_pool(name="tiny", bufs=2))
        dummyp = ctx.enter_context(tc.tile_pool(name="dummy", bufs=4))
        psp = ctx.enter_context(tc.tile_pool(name="ps", bufs=2, space="PSUM"))

        bd_sb = singles.tile([h, h], bf16)
        nc.sync.dma_start(out=bd_sb, in_=bd)
        acc = singles.tile([h, ACC_COLS], f32)
        nc.vector.memset(acc, 0.0)

        nblk = fsz // 512  # matmul free-dim blocks (512 = one PSUM bank)
        planes_per_blk = 512 // w
        nsub = 2 if nblk % 2 == 0 and nblk >= 2 else 1
        hb = nblk // nsub  # psum blocks per half-chunk tile

        def sum_max(out_ap, a_ap, b_ap, col):
            """out = max(a,b); acc[:,col] = sum(out). out is write-only."""
            nc.vector.scalar_tensor_tensor(
                out=out_ap,
                in0=a_ap,
                scalar=0.0,
                in1=b_ap,
                op0=AO.bypass,
                op1=AO.max,
                accum_out=acc[:, col : col + 1],
            )

        def sum_relu(src_ap, col):
            """acc[:,col] = sum(max(src,0)); src rewritten in place."""
            nc.vector.tensor_scalar(
                out=src_ap,
                in0=src_ap,
                scalar1=0.0,
                scalar2=None,
                op0=AO.max,
                op1=AO.add,
                accum_out=acc[:, col : col + 1],
            )

        def sum_ident(src_ap, col):
            """acc[:,col] = sum(src); src rewritten in place (x + 0.0).

            Only used on tiles of non-negative values (x in [0,1)), so the
            identity rewrite is bit-exact.
            """
            nc.vector.tensor_scalar(
                out=src_ap,
                in0=src_ap,
                scalar1=0.0,
                scalar2=None,
                op0=AO.add,
                op1=AO.add,
                accum_out=acc[:, col : col + 1],
            )

        state = {"prev_xb": None, "pending_gy": None, "pending_c": None}

        def emit_gy(ps_tile, cidx, sub):
            dya = dummyp.tile([h, 1], bf16)
            col = COL_DY + nsub * cidx + sub
            nc.scalar.activation(
                out=dya.broadcast_to((h, hb, 512)),
                in_=ps_tile[:, :, :],
                func=AF.Abs,
                accum_out=acc[:, col : col + 1],
            )

        def stage_c(j, half, cidx, xb, xbf):
            """xb-dependent work, emitted one chunk late (see module doc)."""
            # h-diff (gy) via bidiagonal matmul into PSUM; two half-chunk
            # tiles so PE fills one while ScalarE drains the other, and each
            # drain is deferred past the next fill.
            if "gy" not in skip:
                for sub in range(nsub):
                    ps = psp.tile([h, hb, 512], f32)
                    for blk in range(hb):
                        g = sub * hb + blk
                        nc.tensor.matmul(
                            ps[:, blk, :],
                            bd_sb,
                            xb[:, g * planes_per_blk : (g + 1) * planes_per_blk, :],
                            start=True,
                            stop=True,
                        )
                    if state["pending_gy"] is not None:
                        emit_gy(*state["pending_gy"])
                    state["pending_gy"] = (ps, cidx, sub)

            # d-diff (gz), in-chunk pairs: aligned TT subtract (2x) then
            # fused relu-sum (4x); sum(dz) telescopes on host.
            if "dz" not in skip:
                dz = dzp.tile([h, fsz - w], bf16)
                nc.vector.tensor_tensor(
                    out=dz,
                    in0=xbf[:, w:fsz],
                    in1=xbf[:, 0 : fsz - w],
                    op=AO.subtract,
                )
                sum_relu(dz[:, :], COL_DZP + cidx)
                # first/last plane sums for the signed sums
                sum_ident(xb[:, 0, :], COL_TZF + cidx)
                sum_ident(xb[:, ndh - 1, :], COL_TZL + cidx)

                # boundary pair between the two halves of slice j
                if half == 1:
                    bnd = tinyp.tile([h, w], bf16)
                    sum_max(
                        bnd,
                        xb[:, 0, :],
                        state["prev_xb"][:, ndh - 1, :],
                        COL_BNDP + j,
                    )
                state["prev_xb"] = xb

        def chunk_body(j, half):
            cidx = j * 2 + half
            d0 = half * ndh

            # 1) load chunk: [h partitions, ndh planes, w] fp32
            x32 = x32p.tile([h, ndh, w], f32)
            nc.sync.dma_start(
                out=x32, in_=seg[j, d0 : d0 + ndh, :, :].rearrange("d h w -> h d w")
            )

            # 2) cast to bf16; fused accum -> crown/root sum for this chunk
            if "conv" in skip:
                return
            xb = xbp.tile([h, ndh, w], bf16)
            nc.scalar.activation(
                out=xb,
                in_=x32,
                func=AF.Copy,
                accum_out=acc[:, COL_X + cidx : COL_X + cidx + 1],
            )
            xbf = xb.rearrange("p a b -> p (a b)")

            # 3) w-diff (gx): one fused op per chunk.  The exact 3D AP
            #    (misaligned by one element) runs at 1x either way, so it
            #    reads the fp32 tile directly: no dependency on the cast,
            #    and full fp32 precision for the gx term.
            # 4) run the previous chunk's deferred xb-dependent work FIRST:
            #    it is ready now, while this chunk's dx still waits on its
            #    DMA -- this order lets VectorE cover DMA latency
            if state["pending_c"] is not None:
                stage_c(*state["pending_c"])
            state["pending_c"] = (j, half, cidx, xb, xbf)

            if "dx" not in skip:
                dx = dxp.tile([h, ndh, w - 1], bf16)
                sum_max(dx, x32[:, :, 1:], x32[:, :, 0 : w - 1], COL_DXP + cidx)
                # boundary-column sums for the signed sums (fp32)
                sum_ident(x32[:, :, 0:1], COL_TXF + cidx)
                sum_ident(x32[:, :, w - 1 : w], COL_TXL + cidx)

        def all_chunks():
            for j in range(jpc):
                for half in range(2):
                    chunk_body(j, half)
            if state["pending_c"] is not None:
                stage_c(*state["pending_c"])
            state["pending_c"] = None
            if state["pending_gy"] is not None:
                emit_gy(*state["pending_gy"])
            state["pending_gy"] = None

        if repeat == 1:
            all_chunks()
        else:
            with tc.For_i(0, repeat, 1):
                all_chunks()
        nc.sync.dma_start(out=out, in_=acc)

    nc.compile()
    return nc


def _get_program():
    key = "full"
    if key not in _PROG_CACHE:
        _PROG_CACHE[key] = _build_program()
    return _PROG_CACHE[key]


def _bidiag_np(h=H):
    """lhsT for the h-diff matmul: out[m,:] = rhs[m+1,:] - rhs[m,:]."""
    import ml_dtypes

    m = np.zeros((h, h), dtype=np.float32)
    for c in range(h - 1):
        m[c + 1, c] = 1.0
        m[c, c] = -1.0
    # last column stays zero -> output row h-1 is 0
    return m.astype(ml_dtypes.bfloat16)


def _combine(partials, b=B, c=C, d=D, h=H, w=W):
    """Host-side finish: per-core [128, 160] fp32 partials -> [3] fp32."""
    nslice = b * c
    jpc = nslice // len(partials)

    crown = np.zeros(nslice, dtype=np.float64)
    root = np.zeros(nslice, dtype=np.float64)
    gx_sum = 0.0
    gy_sum = 0.0
    gz_sum = 0.0
    for k, p in enumerate(partials):
        p = p.astype(np.float64)
        xp = p[:, COL_DXP : COL_DXP + 2 * jpc].sum(axis=0)
        txf = p[:, COL_TXF : COL_TXF + 2 * jpc].sum(axis=0)
        txl = p[:, COL_TXL : COL_TXL + 2 * jpc].sum(axis=0)
        zp = p[:, COL_DZP : COL_DZP + 2 * jpc].sum(axis=0)
        tzf = p[:, COL_TZF : COL_TZF + 2 * jpc].sum(axis=0)
        tzl = p[:, COL_TZL : COL_TZL + 2 * jpc].sum(axis=0)
        bndp = p[:, COL_BNDP : COL_BNDP + jpc].sum(axis=0)

        xs = p[:, COL_X : COL_X + 2 * jpc].sum(axis=0)
        # sum|a-b| = 2*sum(max(a,b)) - sum(a) - sum(b)
        # gx: a = x[..., 1:], b = x[..., :-1]
        gx_sum += (2.0 * xp - (xs - txf) - (xs - txl)).sum()
        # gz: dz = planes[1:] - planes[:-1]; sum(dz) = tzl - tzf
        gz_sum += (2.0 * zp - (tzl - tzf)).sum()
        # boundary pair: a = half1.plane0, b = half0.plane(ndh-1)
        for jj in range(jpc):
            gz_sum += 2.0 * bndp[jj] - tzf[2 * jj + 1] - tzl[2 * jj]
        gy_sum += p[:, COL_DY : COL_DY + 4 * jpc].sum()

        for jj in range(jpc):
            crown[k * jpc + jj] = p[:, COL_X + 2 * jj].sum()
            root[k * jpc + jj] = p[:, COL_X + 2 * jj + 1].sum()

    total = crown + root
    valid = (total > 0) & (root > 0)
    safe_root = np.where(root > 0, root, 1.0)
    ratio_loss = np.where(valid, (crown / safe_root - EXPECTED_RATIO) ** 2, 0.0)
    cr_loss = ratio_loss.sum() / nslice

    nx = nslice * d * h * (w - 1)
    ny = nslice * d * (h - 1) * w
    nz = nslice * (d - 1) * h * w
    tv = gx_sum / nx + gy_sum / ny + gz_sum / nz

    crown_root = cr_loss * CROWN_ROOT_W
    smoothness = tv * SMOOTH_W
    return np.array(
        [crown_root, smoothness, crown_root + smoothness], dtype=np.float32
    )


def kernel(segmentation: np.ndarray) -> np.ndarray:
    global last_exec_time_ns
    from concourse.bass_utils import run_bass_kernel_spmd

    seg = np.ascontiguousarray(np.asarray(segmentation), dtype=np.float32)
    assert seg.shape == (B, C, D, H, W)
    nc = _get_program()

    bd = _bidiag_np()
    shards = seg.reshape(B * C, D, H, W)
    in_maps = [
        {"seg": np.ascontiguousarray(shards[k * JPC : (k + 1) * JPC]), "bidiag": bd}
        for k in range(NCORES)
    ]
    trace = bool(os.environ.get("BASS_TRACE"))
    res = run_bass_kernel_spmd(nc, in_maps, list(range(NCORES)), trace=trace)
    last_exec_time_ns = res.exec_time_ns
    partials = [res.results[k]["partials"] for k in range(NCORES)]
    return _combine(partials)



# revision 3
# speedup vs baseline: 1.4519x; 1.4519x over previous
"""Trainium2 Bass kernel for nn_DentalAnatomyLoss.

Computes, for segmentation [B=2, C=32, D=64, H=128, W=128] fp32:
  - crown/root ratio loss (per (b,c) sums over d<32 / d>=32)
  - 3D total-variation loss (mean |diff| along w, h, d)
  - returns stack([crown_root, smoothness, total_anatomy]) fp32 [3]

Strategy: pure data-parallel over the 64 (b,c) slices, 8 per NeuronCore.
Each core reduces its 32 MiB shard to a [128, 144] fp32 partial tensor;
the host combines partials into the 3 scalars.

Layout: partition p = 2*d + s where s = h//64, free f = (r, w) with
r = h % 64.  The DMA loads each slice as one [128, 8192] transfer whose
per-partition source is a contiguous 32 KiB block (d-plane half-row),
and casts fp32 -> bf16 in the SDMA datapath (SWDGE), so no engine pass
is spent on the cast and HBM traffic is the fp32 read only.

Per-core engine split (memory regime, ~94 us HBM roofline/core):
  - TensorE: d-diffs (gz) via a block-bidiagonal matmul (the d axis sits
    on partitions); columns 126/127 of the same stationary carry
    crown/ones indicator vectors so the crown/total sums ride along in
    otherwise-zero psum rows.  Two more tiny stationaries compute the
    h=63|64 boundary row diffs (dual accumulated matmuls) and the
    d=0 / d=63 plane sums (for the gz telescoping term).
  - ScalarE: relu+accum drains of all psum tiles.
  - VectorE: h-diffs (gy) as aligned bf16 subtract (2x) + fused
    relu-sum (4x); w-diffs (gx) as one fused max+accum (1x, the
    shift-by-one AP cannot reach a packed mode); tiny row/col sums for
    the telescoping identities sum|a-b| = 2*sum(max/relu) - signed sums.
"""

import os

import numpy as np

B, C, D, H, W = 2, 32, 64, 128, 128
NCORES = 8
JPC = (B * C) // NCORES  # (b,c) slices per core
CROWN_ROOT_W = 2.0
SMOOTH_W = 1.5
EXPECTED_RATIO = 1.2

# accumulator column layout in the [128, ACC_COLS] partial tensor
ACC_COLS = 144
GZ0 = 0      # 64: per gz-psum-tile relu sums (8 tiles x 8 slices);
             #     rows 0..125 = relu(dz), row 126 = crown, row 127 = total
GYR = 64     # 8: sum(relu(dy_internal)) per slice
GYS0 = 72    # 8: per-partition rowsum r=0 per slice
GYS1 = 80    # 8: per-partition rowsum r=63 per slice
GX = 88      # 8: sum(max(x_w, x_{w+1})) per slice
GXC0 = 96    # 8: per-partition colsum w=0 per slice
GXC1 = 104   # 8: per-partition colsum w=127 per slice
PLC = 112    # 8: rows 0/1 = plane d=0 / d=63 sums per slice
PBR = 120    # 8: sum(relu(boundary dy)) per slice
PBS = 128    # 8: sum(boundary dy) per slice
# 136:144 unused (zeroed)

_PROG_CACHE: dict = {}
last_exec_time_ns = None  # set by kernel() when tracing is enabled


def _build_program(jpc=JPC, d=D, h=H, w=W, repeat=1, small_input=False):
    """Build the (single) SPMD Bass program run identically on all cores.

    repeat>1 wraps the whole compute in a hardware For_i loop (identical
    result, used only for wall-clock timing of the kernel body).
    small_input shrinks the dram input to one slice (re-read jpc times)
    so timing runs ship 8x less data through the tunnel.
    """
    from contextlib import ExitStack

    import concourse.tile as tile
    from concourse import bacc, mybir

    f32 = mybir.dt.float32
    bf16 = mybir.dt.bfloat16
    AO = mybir.AluOpType
    AF = mybir.ActivationFunctionType

    assert (d, h, w) == (64, 128, 128), "layout is hardcoded for 64x128x128"
    hh = h // 2          # rows per partition-half (64)
    fsz = hh * w         # free size per partition (8192)
    nblk = fsz // 512    # 512-blocks per slice (16)

    nc = bacc.Bacc(
        "TRN2",
        target_bir_lowering=False,
        debug=False,
        enable_asserts=False,
        num_devices=NCORES,
    )
    jdram = 1 if small_input else jpc
    seg = nc.dram_tensor("seg", [jdram, d, h, w], f32, kind="ExternalInput").ap()
    mats = nc.dram_tensor("mats", [128, 3 * 128 + 2], bf16, kind="ExternalInput").ap()
    out = nc.dram_tensor("partials", [128, ACC_COLS], f32, kind="ExternalOutput").ap()

    with tile.TileContext(nc) as tc, ExitStack() as ctx:
        singles = ctx.enter_context(tc.tile_pool(name="singles", bufs=1))
        xbp = ctx.enter_context(tc.tile_pool(name="xb", bufs=3))
        dyp = ctx.enter_context(tc.tile_pool(name="dy", bufs=2))
        dxp = ctx.enter_context(tc.tile_pool(name="dx", bufs=2))
        tinyp = ctx.enter_context(tc.tile_pool(name="tiny", bufs=2))
        dummyp = ctx.enter_context(tc.tile_pool(name="dummy", bufs=4))
        pszp = ctx.enter_context(tc.tile_pool(name="psz", bufs=2, space="PSUM"))
        psbp = ctx.enter_context(tc.tile_pool(name="psb", bufs=2, space="PSUM"))
        pslp = ctx.enter_context(tc.tile_pool(name="psl", bufs=2, space="PSUM"))

        mats_sb = singles.tile([128, 3 * 128 + 2], bf16)
        nc.sync.dma_start(out=mats_sb, in_=mats)
        Bz = mats_sb[:, 0:128]
        A1 = mats_sb[:, 128:256]
        A2 = mats_sb[:, 256:384]
        PL = mats_sb[:, 384:386]

        acc = singles.tile([128, ACC_COLS], f32)
        nc.vector.memset(acc, 0.0)

        def drain(ps_ap, func, col_ap, fd):
            np_ = ps_ap.shape[0]
            dmy = dummyp.tile([128, 1], bf16)
            nc.scalar.activation(
                out=dmy[0:np_, :].broadcast_to((np_, fd)),
                in_=ps_ap,
                func=func,
                accum_out=col_ap,
            )

        def slice_body(j):
            src = seg[0 if small_input else j]
            xb = xbp.tile([128, fsz], bf16)
            nc.gpsimd.dma_start(
                out=xb, in_=src.rearrange("d (s r) w -> (d s) (r w)", s=2)
            )
            x3 = xb.rearrange("p (r w) -> p r w", w=w)

            # ---- VectorE ----
            # gy internal: dy = x[h+1] - x[h] within each half (aligned 2x),
            # then fused relu+sum (4x).
            dy = dyp.tile([128, fsz - w], bf16)
            nc.vector.tensor_tensor(
                out=dy, in0=xb[:, w:fsz], in1=xb[:, 0 : fsz - w], op=AO.subtract
            )
            nc.vector.tensor_scalar(
                out=dy,
                in0=dy,
                scalar1=0.0,
                scalar2=None,
                op0=AO.max,
                op1=AO.add,
                accum_out=acc[:, GYR + j : GYR + j + 1],
            )
            # rowsums r=0 / r=63 for the gy telescoping term
            t0 = tinyp.tile([128, w], bf16)
            nc.vector.tensor_scalar(
                out=t0, in0=xb[:, 0:w], scalar1=0.0, scalar2=None,
                op0=AO.add, op1=AO.add,
                accum_out=acc[:, GYS0 + j : GYS0 + j + 1],
            )
            t1 = tinyp.tile([128, w], bf16)
            nc.vector.tensor_scalar(
                out=t1, in0=xb[:, fsz - w : fsz], scalar1=0.0, scalar2=None,
                op0=AO.add, op1=AO.add,
                accum_out=acc[:, GYS1 + j : GYS1 + j + 1],
            )
            # gx: fused max+accum over w-adjacent pairs (1x)
            dx = dxp.tile([128, hh, w - 1], bf16)
            nc.vector.scalar_tensor_tensor(
                out=dx,
                in0=x3[:, :, 1:],
                scalar=0.0,
                in1=x3[:, :, 0 : w - 1],
                op0=AO.bypass,
                op1=AO.max,
                accum_out=acc[:, GX + j : GX + j + 1],
            )
            # colsums w=0 / w=127 for the gx telescoping term
            c0 = tinyp.tile([128, hh, 1], bf16)
            nc.vector.tensor_scalar(
                out=c0, in0=x3[:, :, 0:1], scalar1=0.0, scalar2=None,
                op0=AO.add, op1=AO.add,
                accum_out=acc[:, GXC0 + j : GXC0 + j + 1],
            )
            c1 = tinyp.tile([128, hh, 1], bf16)
            nc.vector.tensor_scalar(
                out=c1, in0=x3[:, :, w - 1 : w], scalar1=0.0, scalar2=None,
                op0=AO.add, op1=AO.add,
                accum_out=acc[:, GXC1 + j : GXC1 + j + 1],
            )

            # ---- TensorE + ScalarE ----
            # gz: dz rows q=2d+s = x[d+1] - x[d]; rows 126/127 carry
            # crown/total column sums (>=0, so the relu drain is exact).
            for t in range(nblk // 2):
                ps = pszp.tile([128, 1024], mybir.dt.float32)
                for b2 in range(2):
                    blk = t * 2 + b2
                    nc.tensor.matmul(
                        ps[:, b2 * 512 : (b2 + 1) * 512],
                        Bz,
                        xb[:, blk * 512 : (blk + 1) * 512],
                        start=True,
                        stop=True,
                    )
                col = GZ0 + j * (nblk // 2) + t
                drain(ps, AF.Relu, acc[:, col : col + 1], 1024)

            # plane d=0 / d=63 sums (gz telescoping), accumulated over blocks
            pl = pslp.tile([2, 512], mybir.dt.float32)
            for blk in range(nblk):
                nc.tensor.matmul(
                    pl,
                    PL,
                    xb[:, blk * 512 : (blk + 1) * 512],
                    start=(blk == 0),
                    stop=(blk == nblk - 1),
                )
            drain(pl, AF.Relu, acc[0:2, PLC + j : PLC + j + 1], 512)

            # gy boundary rows h=63|64: pb[2d, :] = x[2d+1, 0:w] - x[2d, fsz-w:]
            pb = psbp.tile([128, w], mybir.dt.float32)
            nc.tensor.matmul(pb, A1, xb[:, 0:w], start=True, stop=False)
            nc.tensor.matmul(pb, A2, xb[:, fsz - w : fsz], start=False, stop=True)
            drain(pb, AF.Relu, acc[:, PBR + j : PBR + j + 1], w)
            drain(pb, AF.Copy, acc[:, PBS + j : PBS + j + 1], w)

        def all_slices():
            for j in range(jpc):
                slice_body(j)

        if repeat == 1:
            all_slices()
        else:
            with tc.For_i(0, repeat, 1):
                all_slices()
        nc.sync.dma_start(out=out, in_=acc)

    nc.compile()
    return nc


def _get_program():
    key = "full"
    if key not in _PROG_CACHE:
        _PROG_CACHE[key] = _build_program()
    return _PROG_CACHE[key]


def _mats_np():
    """Stationary matrices, packed [128, 386] bf16.

    matmul convention: out[q, f] = sum_p lhsT[p, q] * rhs[p, f].
    """
    import ml_dtypes

    m = np.zeros((128, 3 * 128 + 2), dtype=np.float32)
    Bz = m[:, 0:128]
    A1 = m[:, 128:256]
    A2 = m[:, 256:384]
    PL = m[:, 384:386]
    for dd in range(63):
        for s in range(2):
            q = 2 * dd + s
            Bz[2 * dd + 2 + s, q] = 1.0
            Bz[2 * dd + s, q] = -1.0
    Bz[0:64, 126] = 1.0  # crown indicator (d < 32  <->  p < 64)
    Bz[:, 127] = 1.0     # ones (total)
    for dd in range(64):
        A1[2 * dd + 1, 2 * dd] = 1.0
        A2[2 * dd, 2 * dd] = -1.0
    PL[0, 0] = PL[1, 0] = 1.0      # plane d=0  (p in {0,1})
    PL[126, 1] = PL[127, 1] = 1.0  # plane d=63 (p in {126,127})
    return m.astype(ml_dtypes.bfloat16)


def _combine(partials, b=B, c=C, d=D, h=H, w=W):
    """Host-side finish: per-core [128, 144] fp32 partials -> [3] fp32."""
    nslice = b * c
    jpc = nslice // len(partials)
    ntile = 8  # gz psum tiles per slice

    crown = np.zeros(nslice, dtype=np.float64)
    root = np.zeros(nslice, dtype=np.float64)
    gx_sum = 0.0
    gy_sum = 0.0
    gz_sum = 0.0
    for k, p in enumerate(partials):
        p = p.astype(np.float64)
        for j in range(jpc):
            g = slice(GZ0 + j * ntile, GZ0 + (j + 1) * ntile)
            crown_j = p[126, g].sum()
            total_j = p[127, g].sum()
            gzrelu_j = p[0:126, g].sum()
            dzsum_j = p[1, PLC + j] - p[0, PLC + j]
            gz_sum += 2.0 * gzrelu_j - dzsum_j

            gyrelu_j = p[:, GYR + j].sum()
            dysum_j = p[:, GYS1 + j].sum() - p[:, GYS0 + j].sum()
            pbrelu_j = p[:, PBR + j].sum()
            pbsum_j = p[:, PBS + j].sum()
            gy_sum += (2.0 * gyrelu_j - dysum_j) + (2.0 * pbrelu_j - pbsum_j)

            gxmax_j = p[:, GX + j].sum()
            c0_j = p[:, GXC0 + j].sum()
            c1_j = p[:, GXC1 + j].sum()
            gx_sum += 2.0 * gxmax_j - (total_j - c0_j) - (total_j - c1_j)

            crown[k * jpc + j] = crown_j
            root[k * jpc + j] = total_j - crown_j

    total = crown + root
    valid = (total > 0) & (root > 0)
    safe_root = np.where(root > 0, root, 1.0)
    ratio_loss = np.where(valid, (crown / safe_root - EXPECTED_RATIO) ** 2, 0.0)
    cr_loss = ratio_loss.sum() / nslice

    nx = nslice * d * h * (w - 1)
    ny = nslice * d * (h - 1) * w
    nz = nslice * (d - 1) * h * w
    tv = gx_sum / nx + gy_sum / ny + gz_sum / nz

    crown_root = cr_loss * CROWN_ROOT_W
    smoothness = tv * SMOOTH_W
    return np.array(
        [crown_root, smoothness, crown_root + smoothness], dtype=np.float32
    )


def kernel(segmentation: np.ndarray) -> np.ndarray:
    global last_exec_time_ns
    from concourse.bass_utils import run_bass_kernel_spmd

    seg = np.ascontiguousarray(np.asarray(segmentation), dtype=np.float32)
    assert seg.shape == (B, C, D, H, W)
    nc = _get_program()

    mats = _mats_np()
    shards = seg.reshape(B * C, D, H, W)
    in_maps = [
        {"seg": np.ascontiguousarray(shards[k * JPC : (k + 1) * JPC]), "mats": mats}
        for k in range(NCORES)
    ]
    trace = bool(os.environ.get("BASS_TRACE"))
    res = run_bass_kernel_spmd(nc, in_maps, list(range(NCORES)), trace=trace)
    last_exec_time_ns = res.exec_time_ns
    partials = [res.results[k]["partials"] for k in range(NCORES)]
    return _combine(partials)
